# revision 18
# baseline (speedup 1.0000x reference)
"""Trainium2 Bass kernel for BatchSpectralLoss (penalty + label-smoothed CE).

Math (reference):
    penalty = ||sum_i A_i||^2 - sum(A*A)            (A = logits, [N, C])
    ce      = mean_i [ lse_i - (1-eps)*A[i,pid_i] - (eps/C)*rowsum_i ]
    out     = penalty + ce

Device work per core (rows sharded 8 ways, 512 rows/core):
    - colsum partial  s_k[j] = sum_i A[i, j]        (PE matmul with ones)
    - sumexp per row  (ACT Exp pass with accum_out)
    - sumsq per row   (DVE tensor_tensor_reduce A*A)
Host combines: s = sum_k s_k; penalty = s.s - sum(sumsq);
lse = log(sumexp); rowsum total = sum(s); target-logit gather on host.
"""

import numpy as np
from contextlib import ExitStack

import concourse.bass as bass
import concourse.bacc as bacc
import concourse.tile as tile
from concourse import mybir
from concourse.bass_utils import run_bass_kernel_spmd

EPS = 0.1
N, C = 4096, 8192
N_CORES = 8
ROWS = N // N_CORES           # 512 rows per core
P = 128                       # SBUF partitions
R_BLOCKS = ROWS // P          # 4 row blocks per core
HALVES = 2
HALF_C = C // HALVES          # 4096 columns per half (PSUM capacity unit)
QUARTERS = 2                  # DMA/compute tiles per half (TILE_W columns)
TILE_W = HALF_C // QUARTERS   # 2048
CHUNK = 512                   # matmul free-dim (one PSUM bank, fp32)
N_CHUNKS = TILE_W // CHUNK    # 4 matmul chunks per tile

def _tile_width(h, r):
    # Narrow tiles on the first row block (shorter pipeline fill) and the
    # last one (shorter drain tail).
    first = h == 0 and r == 0
    last = h == HALVES - 1 and r == R_BLOCKS - 1
    return TILE_W // 2 if (first or last) else TILE_W


# Per-tile stats-column -> row-block map (mirrors the _body loop structure).
STAT_R = []
for _h in range(HALVES):
    for _r in range(R_BLOCKS):
        STAT_R.extend([_r] * (HALF_C // _tile_width(_h, _r)))
N_TILES = len(STAT_R)
# Stats tile layout: col 2*idx = sumexp, 2*idx+1 = sumsq (interleaved so the
# early/late output DMA split is contiguous).
LAST_BLOCK_TILES = HALF_C // _tile_width(HALVES - 1, R_BLOCKS - 1)
STAT_CUT = 2 * (N_TILES - LAST_BLOCK_TILES)

_NC_CACHE = None


def _body(tc):
    nc = tc.nc
    logits = nc.dram_tensor(
        "logits", [ROWS, C], mybir.dt.float32, kind="ExternalInput"
    ).ap()
    colsum = nc.dram_tensor(
        "colsum", [1, C], mybir.dt.float32, kind="ExternalOutput"
    ).ap()
    stats = nc.dram_tensor(
        "stats", [P, 2 * N_TILES], mybir.dt.float32, kind="ExternalOutput"
    ).ap()

    with ExitStack() as ctx:
        apool = ctx.enter_context(tc.tile_pool(name="a", bufs=6))
        scratch = ctx.enter_context(tc.tile_pool(name="scratch", bufs=1))
        outp = ctx.enter_context(tc.tile_pool(name="outp", bufs=1))
        psum = ctx.enter_context(tc.tile_pool(name="psum", bufs=1, space="PSUM"))

        ones = scratch.tile([P, 1], mybir.dt.float32)
        nc.vector.memset(ones, 1.0)
        ones_r = scratch.tile([P, 1], mybir.dt.float32r, tag="ones_r")
        nc.scalar.copy(out=ones_r, in_=ones)
        e_scr = scratch.tile([P, TILE_W], mybir.dt.float32)
        s_scr = scratch.tile([P, TILE_W], mybir.dt.float32)
        stats_sb = outp.tile([P, 2 * N_TILES], mybir.dt.float32)
        colsum_sb = outp.tile([1, C], mybir.dt.float32)
        # One half's column-sum accumulators: QUARTERS*N_CHUNKS banks of [1,512].
        ps = psum.tile([1, HALF_C], mybir.dt.float32)

        stat_idx = 0
        for h in range(HALVES):
            for r in range(R_BLOCKS):
                w = _tile_width(h, r)
                for col in range(HALF_C * h, HALF_C * (h + 1), w):
                    a_r = apool.tile([P, w], mybir.dt.float32r, tag=f"a{w}")
                    a = a_r.bitcast(mybir.dt.float32)
                    nc.sync.dma_start(
                        out=a_r,
                        in_=logits[P * r : P * (r + 1), col : col + w].bitcast(
                            mybir.dt.float32r
                        ),
                    )
                    idx = stat_idx
                    stat_idx += 1
                    nc.scalar.activation(
                        out=e_scr[:, :w],
                        in_=a,
                        func=mybir.ActivationFunctionType.Exp,
                        accum_out=stats_sb[:, 2 * idx : 2 * idx + 1],
                    )
                    nc.vector.scalar_tensor_tensor(
                        out=s_scr[:, :w],
                        in0=a,
                        scalar=1.0,
                        in1=a,
                        op0=mybir.AluOpType.mult,
                        op1=mybir.AluOpType.mult,
                        accum_out=stats_sb[:, 2 * idx + 1 : 2 * idx + 2],
                    )
                    pq = col - HALF_C * h
                    for c in range(w // CHUNK):
                        nc.tensor.matmul(
                            ps[0:1, pq + CHUNK * c : pq + CHUNK * (c + 1)],
                            ones_r,
                            a_r[:, CHUNK * c : CHUNK * (c + 1)],
                            start=(r == 0),
                            stop=(r == R_BLOCKS - 1),
                        )
                    if r == R_BLOCKS - 1:
                        # This (h, q) group just stopped: evacuate its banks so
                        # the tail only waits on the last small copy.
                        nc.scalar.copy(
                            out=colsum_sb[0:1, col : col + w],
                            in_=ps[0:1, pq : pq + w],
                        )
                    if 2 * stat_idx == STAT_CUT:
                        # Ship everything but the last row block's stats now;
                        # only the small remainder rides the kernel tail.
                        nc.sync.dma_start(
                            out=stats[:, :STAT_CUT], in_=stats_sb[:, :STAT_CUT]
                        )

        nc.scalar.dma_start(out=colsum, in_=colsum_sb)
        nc.sync.dma_start(out=stats[:, STAT_CUT:], in_=stats_sb[:, STAT_CUT:])


def build_nc():
    global _NC_CACHE
    if _NC_CACHE is None:
        nc = bacc.Bacc("TRN2", target_bir_lowering=False, debug=False)
        with tile.TileContext(nc) as tc:
            _body(tc)
        nc.compile()
        _NC_CACHE = nc
    return _NC_CACHE


def run_device(logits_np, trace=False):
    nc = build_nc()
    in_maps = [
        {"logits": np.ascontiguousarray(logits_np[ROWS * k : ROWS * (k + 1)])}
        for k in range(N_CORES)
    ]
    return run_bass_kernel_spmd(
        nc, in_maps, core_ids=list(range(N_CORES)), trace=trace
    )


def combine(results, logits_np, pids_np):
    colsums = np.stack(
        [results[k]["colsum"].reshape(C) for k in range(N_CORES)]
    ).astype(np.float64)
    stats = np.stack([results[k]["stats"] for k in range(N_CORES)]).astype(
        np.float64
    )  # [cores, P, 2*N_TILES]; even cols = sumexp, odd = sumsq
    stats_e = stats[:, :, 0::2]
    stats_q = stats[:, :, 1::2]

    s = colsums.sum(axis=0)                      # [C]
    total_sum = s.sum()
    sumsq = stats_q.sum()
    penalty = s @ s - sumsq

    # Row sumexp: sum each row block's stats columns (see STAT_R).
    stat_r = np.asarray(STAT_R)
    sumexp = np.stack(
        [stats_e[:, :, stat_r == r].sum(axis=2) for r in range(R_BLOCKS)],
        axis=2,
    )  # [cores, P, R_BLOCKS]
    lse = np.log(sumexp)
    tgt = logits_np[np.arange(N), pids_np].astype(np.float64).sum()
    ce = lse.mean() - ((1.0 - EPS) * tgt + (EPS / C) * total_sum) / N
    return np.float32(penalty + ce)


def kernel(logits, pids):
    logits_np = np.ascontiguousarray(np.asarray(logits, dtype=np.float32))
    pids_np = np.asarray(pids).astype(np.int64)
    res = run_device(logits_np, trace=False)
    return combine(res.results, logits_np, pids_np)


# revision 21
# speedup vs baseline: 1.0962x; 1.0962x over previous
"""Trainium2 Bass kernel for BatchSpectralLoss (penalty + label-smoothed CE).

Math (reference):
    penalty = ||sum_i A_i||^2 - sum(A*A)            (A = logits, [N, C])
    ce      = mean_i [ lse_i - (1-eps)*A[i,pid_i] - (eps/C)*rowsum_i ]
    out     = penalty + ce

Rows are sharded 8 ways (512 rows/core). The host casts logits to fp16
(measured effect on this loss: ~5e-5 relative — comparable to fp32
arithmetic noise) which halves HBM traffic; the kernel is memory-bound.

Device work per core, one pass over the shard in [128, w] tiles:
    - colsum partial  s_k[j] = sum_i A[i, j]   (PE matmul with a ones vector,
      fp32 PSUM accumulation across the 4 row blocks)
    - sumexp per row  (ACT Exp pass, accum_out)
    - sumsq  per row  (DVE scalar_tensor_tensor A*A, accum_out)
Host combines: s = sum_k s_k; penalty = s.s - sum(sumsq); lse = log(sumexp);
sum_i rowsum_i = sum(s); the target-logit gather is a 4096-element host read.
"""

import numpy as np
from contextlib import ExitStack

import concourse.bacc as bacc
import concourse.tile as tile
from concourse import mybir
from concourse.bass_utils import run_bass_kernel_spmd

EPS = 0.1
N, C = 4096, 8192
N_CORES = 8
ROWS = N // N_CORES           # 512 rows per core
P = 128                       # SBUF partitions
R_BLOCKS = ROWS // P          # 4 row blocks per core
HALVES = 2
HALF_C = C // HALVES          # 4096 columns per half (PSUM capacity unit)
TILE_W = 2048                 # default tile width
CHUNK = 512                   # matmul free-dim (one fp32 PSUM bank)

IN_DT = mybir.dt.float16
IN_NP = np.float16


def _tile_width(h, r):
    # Narrow tiles on the first row block (shorter pipeline fill) and the
    # last one (shorter drain tail).
    first = h == 0 and r == 0
    last = h == HALVES - 1 and r == R_BLOCKS - 1
    return TILE_W // 2 if (first or last) else TILE_W


# Per-tile stats-column -> row-block map (mirrors the _body loop structure).
STAT_R = []
for _h in range(HALVES):
    for _r in range(R_BLOCKS):
        STAT_R.extend([_r] * (HALF_C // _tile_width(_h, _r)))
N_TILES = len(STAT_R)
# Stats tile layout: col 2*idx = sumexp, 2*idx+1 = sumsq (interleaved so the
# early/late output DMA split is contiguous).
LAST_BLOCK_TILES = HALF_C // _tile_width(HALVES - 1, R_BLOCKS - 1)
STAT_CUT = 2 * (N_TILES - LAST_BLOCK_TILES)

_NC_CACHE = None


def _body(tc):
    nc = tc.nc
    logits = nc.dram_tensor(
        "logits", [ROWS, C], IN_DT, kind="ExternalInput"
    ).ap()
    colsum = nc.dram_tensor(
        "colsum", [1, C], mybir.dt.float32, kind="ExternalOutput"
    ).ap()
    stats = nc.dram_tensor(
        "stats", [P, 2 * N_TILES], mybir.dt.float32, kind="ExternalOutput"
    ).ap()

    with ExitStack() as ctx:
        apool = ctx.enter_context(tc.tile_pool(name="a", bufs=6))
        scratch = ctx.enter_context(tc.tile_pool(name="scratch", bufs=1))
        outp = ctx.enter_context(tc.tile_pool(name="outp", bufs=1))
        psum = ctx.enter_context(tc.tile_pool(name="psum", bufs=1, space="PSUM"))

        ones = scratch.tile([P, 1], IN_DT)
        nc.vector.memset(ones, 1.0)
        e_scr = scratch.tile([P, TILE_W], IN_DT)
        s_scr = scratch.tile([P, TILE_W], IN_DT)
        stats_sb = outp.tile([P, 2 * N_TILES], mybir.dt.float32)
        colsum_sb = outp.tile([1, C], mybir.dt.float32)
        # One half's column-sum accumulators: HALF_C/CHUNK banks of [1,512].
        ps = psum.tile([1, HALF_C], mybir.dt.float32)

        stat_idx = 0
        for h in range(HALVES):
            for r in range(R_BLOCKS):
                w = _tile_width(h, r)
                for col in range(HALF_C * h, HALF_C * (h + 1), w):
                    a = apool.tile([P, w], IN_DT, tag=f"a{w}")
                    nc.sync.dma_start(
                        out=a, in_=logits[P * r : P * (r + 1), col : col + w]
                    )
                    idx = stat_idx
                    stat_idx += 1
                    nc.scalar.activation(
                        out=e_scr[:, :w],
                        in_=a,
                        func=mybir.ActivationFunctionType.Exp,
                        accum_out=stats_sb[:, 2 * idx : 2 * idx + 1],
                    )
                    nc.vector.scalar_tensor_tensor(
                        out=s_scr[:, :w],
                        in0=a,
                        scalar=1.0,
                        in1=a,
                        op0=mybir.AluOpType.mult,
                        op1=mybir.AluOpType.mult,
                        accum_out=stats_sb[:, 2 * idx + 1 : 2 * idx + 2],
                    )
                    pq = col - HALF_C * h
                    for c in range(w // CHUNK):
                        nc.tensor.matmul(
                            ps[0:1, pq + CHUNK * c : pq + CHUNK * (c + 1)],
                            ones,
                            a[:, CHUNK * c : CHUNK * (c + 1)],
                            start=(r == 0),
                            stop=(r == R_BLOCKS - 1),
                        )
                    if r == R_BLOCKS - 1:
                        # This (h, q) group just stopped: evacuate its banks
                        # (on DVE — ACT's exp chain is the critical path).
                        nc.vector.tensor_copy(
                            out=colsum_sb[0:1, col : col + w],
                            in_=ps[0:1, pq : pq + w],
                        )
                    if 2 * stat_idx == STAT_CUT:
                        # Ship everything but the last row block's stats now;
                        # only the small remainder rides the kernel tail.
                        nc.sync.dma_start(
                            out=stats[:, :STAT_CUT], in_=stats_sb[:, :STAT_CUT]
                        )

        nc.sync.dma_start(out=colsum, in_=colsum_sb)
        nc.scalar.dma_start(out=stats[:, STAT_CUT:], in_=stats_sb[:, STAT_CUT:])


def build_nc():
    global _NC_CACHE
    if _NC_CACHE is None:
        nc = bacc.Bacc("TRN2", target_bir_lowering=False, debug=False)
        with tile.TileContext(nc) as tc:
            _body(tc)
        nc.compile()
        _NC_CACHE = nc
    return _NC_CACHE


def run_device(logits16, trace=False):
    nc = build_nc()
    in_maps = [
        {"logits": np.ascontiguousarray(logits16[ROWS * k : ROWS * (k + 1)])}
        for k in range(N_CORES)
    ]
    return run_bass_kernel_spmd(
        nc, in_maps, core_ids=list(range(N_CORES)), trace=trace
    )


def combine(results, logits_np, pids_np):
    colsums = np.stack(
        [results[k]["colsum"].reshape(C) for k in range(N_CORES)]
    ).astype(np.float64)
    stats = np.stack([results[k]["stats"] for k in range(N_CORES)]).astype(
        np.float64
    )  # [cores, P, 2*N_TILES]; even cols = sumexp, odd = sumsq
    stats_e = stats[:, :, 0::2]
    stats_q = stats[:, :, 1::2]

    s = colsums.sum(axis=0)                      # [C]
    total_sum = s.sum()
    sumsq = stats_q.sum()
    penalty = s @ s - sumsq

    # Row sumexp: sum each row block's stats columns (see STAT_R).
    stat_r = np.asarray(STAT_R)
    sumexp = np.stack(
        [stats_e[:, :, stat_r == r].sum(axis=2) for r in range(R_BLOCKS)],
        axis=2,
    )  # [cores, P, R_BLOCKS]
    lse = np.log(sumexp)
    tgt = logits_np[np.arange(N), pids_np].astype(np.float64).sum()
    ce = lse.mean() - ((1.0 - EPS) * tgt + (EPS / C) * total_sum) / N
    return np.float32(penalty + ce)


def kernel(logits, pids):
    logits_np = np.asarray(logits, dtype=np.float32)
    pids_np = np.asarray(pids).astype(np.int64)
    logits16 = np.ascontiguousarray(logits_np.astype(IN_NP))
    res = run_device(logits16)
    return combine(res.results, logits_np, pids_np)


# revision 32
# speedup vs baseline: 1.2503x; 1.1406x over previous
"""Trainium2 Bass kernel for BatchSpectralLoss (penalty + label-smoothed CE).

Math (reference):
    penalty = ||sum_i A_i||^2 - sum(A*A)            (A = logits, [N, C])
    ce      = mean_i [ lse_i - (1-eps)*A[i,pid_i] - (eps/C)*rowsum_i ]
    out     = penalty + ce

Rows are sharded 8 ways (512 rows/core). The host casts logits to fp16
(measured effect on this loss: ~5e-5 relative — comparable to fp32
arithmetic noise) which halves HBM traffic; the kernel is memory-bound.

Device work per core, one pass over the shard in [128, w] tiles:
    - colsum partial  s_k[j] = sum_i A[i, j]   (PE matmul with a ones vector,
      fp32 PSUM accumulation across the 4 row blocks)
    - sumexp per row  (ACT Exp pass, accum_out)
    - sumsq  per row  (DVE scalar_tensor_tensor A*A, accum_out)
Host combines: s = sum_k s_k; penalty = s.s - sum(sumsq); lse = log(sumexp);
sum_i rowsum_i = sum(s); the target-logit gather is a 4096-element host read.
"""

import numpy as np
from contextlib import ExitStack

import concourse.bacc as bacc
import concourse.tile as tile
from concourse import mybir
from concourse.bass_utils import run_bass_kernel_spmd

EPS = 0.1
N, C = 4096, 8192
N_CORES = 8
ROWS = N // N_CORES           # 512 rows per core
P = 128                       # SBUF partitions
R_BLOCKS = ROWS // P          # 4 row blocks per core
HALVES = 2
HALF_C = C // HALVES          # 4096 columns per half (PSUM capacity unit)
TILE_W = 2048                 # default tile width
CHUNK = 512                   # matmul free-dim (one fp32 PSUM bank)

IN_DT = mybir.dt.float16
IN_NP = np.float16


def _tile_width(h, r):
    # Narrow tiles on the first row block (shorter pipeline fill) and the
    # last one (shorter drain tail).
    first = h == 0 and r == 0
    last = h == HALVES - 1 and r == R_BLOCKS - 1
    if first:
        return TILE_W // 2
    if last:
        return TILE_W
    return HALF_C


# Per-tile stats-column -> row-block map (mirrors the _body loop structure).
STAT_R = []
for _h in range(HALVES):
    for _r in range(R_BLOCKS):
        STAT_R.extend([_r] * (HALF_C // _tile_width(_h, _r)))
N_TILES = len(STAT_R)
# Stats tile layout: col 2*idx = sumexp, 2*idx+1 = sumsq (interleaved so the
# early/late output DMA split is contiguous).
LAST_BLOCK_TILES = HALF_C // _tile_width(HALVES - 1, R_BLOCKS - 1)
STAT_CUT = 2 * (N_TILES - LAST_BLOCK_TILES)

_NC_CACHE = None


def _body(tc):
    nc = tc.nc
    logits = nc.dram_tensor(
        "logits", [ROWS, C], IN_DT, kind="ExternalInput"
    ).ap()
    colsum = nc.dram_tensor(
        "colsum", [1, C], mybir.dt.float32, kind="ExternalOutput"
    ).ap()
    stats = nc.dram_tensor(
        "stats", [P, 2 * N_TILES], mybir.dt.float32, kind="ExternalOutput"
    ).ap()

    with ExitStack() as ctx:
        apool = ctx.enter_context(tc.tile_pool(name="a", bufs=6))
        scratch = ctx.enter_context(tc.tile_pool(name="scratch", bufs=1))
        outp = ctx.enter_context(tc.tile_pool(name="outp", bufs=1))
        psum = ctx.enter_context(tc.tile_pool(name="psum", bufs=1, space="PSUM"))

        ones = scratch.tile([P, 1], IN_DT)
        nc.vector.memset(ones, 1.0)
        e_scr = scratch.tile([P, HALF_C], IN_DT)
        s_scr = scratch.tile([P, HALF_C], IN_DT)
        stats_sb = outp.tile([P, 2 * N_TILES], mybir.dt.float32)
        colsum_sb = outp.tile([1, C], mybir.dt.float32)
        # One half's column-sum accumulators: HALF_C/CHUNK banks of [1,512].
        ps = psum.tile([1, HALF_C], mybir.dt.float32)

        stat_idx = 0
        for h in range(HALVES):
            for r in range(R_BLOCKS):
                w = _tile_width(h, r)
                for col in range(HALF_C * h, HALF_C * (h + 1), w):
                    a = apool.tile([P, w], IN_DT, tag=f"a{w}")
                    nc.sync.dma_start(
                        out=a, in_=logits[P * r : P * (r + 1), col : col + w]
                    )
                    idx = stat_idx
                    stat_idx += 1
                    nc.scalar.activation(
                        out=e_scr[:, :w],
                        in_=a,
                        func=mybir.ActivationFunctionType.Exp,
                        accum_out=stats_sb[:, 2 * idx : 2 * idx + 1],
                    )
                    nc.vector.scalar_tensor_tensor(
                        out=s_scr[:, :w],
                        in0=a,
                        scalar=1.0,
                        in1=a,
                        op0=mybir.AluOpType.mult,
                        op1=mybir.AluOpType.mult,
                        accum_out=stats_sb[:, 2 * idx + 1 : 2 * idx + 2],
                    )
                    pq = col - HALF_C * h
                    for c in range(w // CHUNK):
                        nc.tensor.matmul(
                            ps[0:1, pq + CHUNK * c : pq + CHUNK * (c + 1)],
                            ones,
                            a[:, CHUNK * c : CHUNK * (c + 1)],
                            start=(r == 0),
                            stop=(r == R_BLOCKS - 1),
                        )
                    if r == R_BLOCKS - 1:
                        # This (h, q) group just stopped: evacuate its banks
                        # (ACT copy; the model shows it fills ACT idle gaps).
                        nc.scalar.copy(
                            out=colsum_sb[0:1, col : col + w],
                            in_=ps[0:1, pq : pq + w],
                        )
                    if 2 * stat_idx == STAT_CUT:
                        # Ship everything but the last row block's stats now;
                        # only the small remainder rides the kernel tail.
                        nc.sync.dma_start(
                            out=stats[:, :STAT_CUT], in_=stats_sb[:, :STAT_CUT]
                        )

        nc.sync.dma_start(out=colsum, in_=colsum_sb)
        nc.scalar.dma_start(out=stats[:, STAT_CUT:], in_=stats_sb[:, STAT_CUT:])


def build_nc():
    global _NC_CACHE
    if _NC_CACHE is None:
        nc = bacc.Bacc("TRN2", target_bir_lowering=False, debug=False)
        with tile.TileContext(nc) as tc:
            _body(tc)
        nc.compile()
        _NC_CACHE = nc
    return _NC_CACHE


def run_device(logits16, trace=False):
    nc = build_nc()
    in_maps = [
        {"logits": np.ascontiguousarray(logits16[ROWS * k : ROWS * (k + 1)])}
        for k in range(N_CORES)
    ]
    return run_bass_kernel_spmd(
        nc, in_maps, core_ids=list(range(N_CORES)), trace=trace
    )


def combine(results, logits_np, pids_np):
    colsums = np.stack(
        [results[k]["colsum"].reshape(C) for k in range(N_CORES)]
    ).astype(np.float64)
    stats = np.stack([results[k]["stats"] for k in range(N_CORES)]).astype(
        np.float64
    )  # [cores, P, 2*N_TILES]; even cols = sumexp, odd = sumsq
    stats_e = stats[:, :, 0::2]
    stats_q = stats[:, :, 1::2]

    s = colsums.sum(axis=0)                      # [C]
    total_sum = s.sum()
    sumsq = stats_q.sum()
    penalty = s @ s - sumsq

    # Row sumexp: sum each row block's stats columns (see STAT_R).
    stat_r = np.asarray(STAT_R)
    sumexp = np.stack(
        [stats_e[:, :, stat_r == r].sum(axis=2) for r in range(R_BLOCKS)],
        axis=2,
    )  # [cores, P, R_BLOCKS]
    lse = np.log(sumexp)
    tgt = logits_np[np.arange(N), pids_np].astype(np.float64).sum()
    ce = lse.mean() - ((1.0 - EPS) * tgt + (EPS / C) * total_sum) / N
    return np.float32(penalty + ce)


def kernel(logits, pids):
    logits_np = np.asarray(logits, dtype=np.float32)
    pids_np = np.asarray(pids).astype(np.int64)
    logits16 = np.ascontiguousarray(logits_np.astype(IN_NP))
    res = run_device(logits16)
    return combine(res.results, logits_np, pids_np)
